# revision 38
# baseline (speedup 1.0000x reference)
"""Trainium2 Bass kernel for local-window sparse attention.

Problem (hardcoded):
  x [4, 2048, 512] fp32; qkv proj [512, 1536] + bias; 8 heads, head_dim 64;
  2D local attention on a 16x128 grid with a 7x11 window; out proj [512, 512].

Sharding: 8 cores = 4 batches x 2 head-groups (4 heads each).

Layout trick: queries/keys are permuted to w-major order (idx = w*16 + h) on
the host. Then a 128-query block = 8 w-columns x 16 h-rows, and each query
block attends only key blocks {j-1, j, j+1} (instead of 7 of 16 in h-major
order), and the additive local mask is a single shared [128, 3*128] strip.

Dataflow per core (all matmuls float32r ~= tf32):
  qk^T [512, 2048] = W_qk^T-stationary matmuls over x^T (host-pretransposed)
  V natural [2048, 4*65] (ones column appended per head for rowsums)
  S^T_j [128k, <=384q] = mask-inject (identity matmul) + K_j^T .T @ q^T
  P^T = exp(S^T)               (ScalarE; masked entries underflow to 0)
  outT_aug [65, 512] = sum_j V_aug_j.T @ P^T_j   (row 64 = softmax denoms)
  outT_norm = outT_aug[0:64] * bcast(1/rowsum)   (PE bcast + DVE recip/mult)
  final [128n, 512] = sum_heads outT_norm.T @ W_proj_head  -> DMA out

Host: sums the two head-group partials per batch, adds b_proj + b_v @ w_proj
(v-bias commutes through softmax), and un-permutes rows.
"""

import os

if os.environ.get("JAX_PLATFORMS") == "cpu":
    # The bass kernel needs the neuron/axon jax platform.
    os.environ["JAX_PLATFORMS"] = ""

import numpy as np

DIM = 512
N_HEADS = 8
HEAD_DIM = 64
H, W = 16, 128
HK, WK = 7, 11
B = 4
N = H * W  # 2048
SCALE = HEAD_DIM ** -0.5

N_CORES = 8
HPC = 4  # heads per core
GC = HPC * HEAD_DIM  # 256 feature cols per group
NEG = -60000.0

# w-major permutation: new = w*16 + h  for old = h*128 + w
_old = np.arange(N)
NEW_OF_OLD = (_old % W) * H + (_old // W)
_new = np.arange(N)
OLD_OF_NEW = (_new % H) * W + (_new // H)

GROUPS = [(0, 4), (4, 4), (8, 4), (12, 4)]  # (first chunk, width)

_NC = None  # cached compiled Bass module


def _make_mask_strip() -> np.ndarray:
    """strip[k_local, 128*(r+1) + q_local] for key block j vs query chunk j+r."""
    loc = np.arange(128)
    wl, hl = loc // H, loc % H
    strip = np.full((128, 384), NEG, np.float32)
    for r in (-1, 0, 1):
        dh = hl[:, None] - hl[None, :]
        dw = wl[:, None] - wl[None, :] - 8 * r  # w_k - w_q
        ok = (np.abs(dh) <= HK // 2) & (np.abs(dw) <= WK // 2)
        strip[:, 128 * (r + 1):128 * (r + 2)] = np.where(ok, 0.0, NEG)
    return strip


def _build():
    import concourse.bass as bass
    import concourse.tile as tile
    from concourse import bacc, mybir

    f32 = mybir.dt.float32
    f32r = mybir.dt.float32r
    Exp = mybir.ActivationFunctionType.Exp

    nc = bacc.Bacc("TRN2", target_bir_lowering=False, debug=False)

    xT_d = nc.dram_tensor("xT", [DIM, N], f32r, kind="ExternalInput")
    wqk_d = nc.dram_tensor("wqk", [DIM, 512], f32r, kind="ExternalInput")
    wv_d = nc.dram_tensor("wv", [DIM, GC], f32r, kind="ExternalInput")
    wp_d = nc.dram_tensor("wp", [GC, DIM], f32r, kind="ExternalInput")
    bias_d = nc.dram_tensor("bias", [128, 4], f32, kind="ExternalInput")
    mask_d = nc.dram_tensor("mask", [128, 384], f32r, kind="ExternalInput")
    ident_d = nc.dram_tensor("ident", [128, 128], f32r, kind="ExternalInput")
    ones_d = nc.dram_tensor("ones", [128, 64], f32r, kind="ExternalInput")
    out_d = nc.dram_tensor("out", [N, DIM], f32, kind="ExternalOutput")

    with tile.TileContext(nc) as tc:
        with (
            tc.tile_pool(name="const", bufs=1) as cpool,
            tc.tile_pool(name="big", bufs=1) as bpool,
            tc.tile_pool(name="pexp", bufs=22) as ppool,
            tc.tile_pool(name="onorm", bufs=10) as npool,
            tc.tile_pool(name="small", bufs=6) as spool,
            tc.tile_pool(name="outsb", bufs=4) as opool,
            tc.tile_pool(name="psA", bufs=2, space="PSUM") as psA,
            tc.tile_pool(name="psS", bufs=3, space="PSUM") as psS,
        ):
            # ---- constants / inputs to SBUF (ordered to unblock compute) ----
            wqk = [cpool.tile([128, 512], f32r, tag=f"wqk{d}", name=f"wqk{d}") for d in range(4)]
            wv = [cpool.tile([128, GC], f32r, tag=f"wv{d}", name=f"wv{d}") for d in range(4)]
            wp = [cpool.tile([128, 512], f32r, tag=f"wp{t}", name=f"wp{t}") for t in range(2)]
            xT = [cpool.tile([128, N], f32r, tag=f"xT{d}", name=f"xT{d}") for d in range(4)]
            bias = cpool.tile([128, 4], f32, tag="bias")
            mask = cpool.tile([128, 384], f32r, tag="mask")
            ident = cpool.tile([128, 128], f32r, tag="ident")
            onesb = cpool.tile([128, 64], f32r, tag="onesb")

            # single-ring, just-in-time order: each x^T chunk lands right
            # before its QKV compute needs it (chunk compute ~5us, chunk DMA
            # ~2.8us, so the stream stays ahead after the first chunk)
            for d in range(4):
                nc.sync.dma_start(wqk[d][:], wqk_d[128 * d:128 * (d + 1), :])

            def dma_xt(nchk):
                for d in range(4):
                    nc.sync.dma_start(
                        xT[d][:, 512 * nchk:512 * (nchk + 1)],
                        xT_d[128 * d:128 * (d + 1), 512 * nchk:512 * (nchk + 1)],
                    )

            dma_xt(0)
            nc.sync.dma_start(bias[:], bias_d[:])
            nc.sync.dma_start(mask[:], mask_d[:])
            nc.sync.dma_start(ident[:], ident_d[:])
            nc.sync.dma_start(onesb[:], ones_d[:])
            dma_xt(1)
            for d in range(4):
                nc.sync.dma_start(wv[d][:], wv_d[128 * d:128 * (d + 1), :])
            dma_xt(2)
            for t in range(2):
                nc.sync.dma_start(wp[t][:], wp_d[128 * t:128 * (t + 1), :])
            dma_xt(3)

            # ---- phase B+C interleaved per n-chunk: qk^T and V ----
            qk = [bpool.tile([128, N], f32r, tag=f"qk{t}", name=f"qk{t}") for t in range(4)]
            V = bpool.tile([128, 16 * HPC * 65], f32r, tag="V")
            Vv = V[:].rearrange("p (j h c) -> p j h c", j=16, h=HPC)
            nc.sync.dma_start(
                Vv[:, :, :, 64],
                ones_d[:].rearrange("p (j h) -> p j h", j=16),
            )
            with nc.named_scope("qkv"):
                for nchk in range(4):
                    for ot in range(4):
                        ps = psA.tile([128, 512], f32, tag="a", name="psa")
                        for d in range(4):
                            nc.tensor.matmul(
                                ps[:],
                                wqk[d][:, 128 * ot:128 * (ot + 1)],
                                xT[d][:, 512 * nchk:512 * (nchk + 1)],
                                start=(d == 0),
                                stop=(d == 3),
                            )
                        nc.vector.tensor_scalar_add(
                            qk[ot][:, 512 * nchk:512 * (nchk + 1)],
                            ps[:],
                            bias[:, ot:ot + 1],
                        )
                    for nt in range(4 * nchk, 4 * nchk + 4):
                        ps = psA.tile([128, GC], f32, tag="a", name="psv",
                                      padded_shape=[128, 512])
                        for d in range(4):
                            nc.tensor.matmul(
                                ps[:],
                                xT[d][:, 128 * nt:128 * (nt + 1)],
                                wv[d][:],
                                start=(d == 0),
                                stop=(d == 3),
                            )
                        nc.vector.tensor_copy(
                            Vv[:, nt, :, 0:64],
                            ps[:].rearrange("p (h c) -> p h c", h=HPC),
                        )

            # helpers for head slicing in qk tiles
            def qT(i):  # [64, N] q^T rows of head i
                return qk[i // 2][64 * (i % 2):64 * (i % 2) + 64, :]

            def kT(i):
                return qk[2 + i // 2][64 * (i % 2):64 * (i % 2) + 64, :]

            P = {i: {} for i in range(HPC)}

            def emit_scores(i, j):
                qlo, qhi = max(0, j - 1), min(15, j + 1)
                wj = 128 * (qhi - qlo + 1)
                mlo = 128 * (qlo - (j - 1))
                ps = psS.tile([128, wj], f32, tag="s", name="pss")
                nc.tensor.matmul(
                    ps[:], ident[:], mask[:, mlo:mlo + wj],
                    start=True, stop=False, skip_group_check=True,
                )
                nc.tensor.matmul(
                    ps[:],
                    kT(i)[:, 128 * j:128 * (j + 1)],
                    qT(i)[:, 128 * qlo:128 * (qhi + 1)],
                    start=False, stop=True, skip_group_check=True,
                )
                pt = ppool.tile([128, wj], f32r, tag="p", name="pt")
                nc.scalar.activation(pt[:], ps[:], Exp)
                P[i][j] = pt

            # ---- t-major: scores -> AV -> normalize -> proj per chunk group
            out_norm = {}  # (pair, group) -> [128, 4*gw] tile
            for t, (c0, gw) in enumerate(GROUPS):
                c1 = c0 + gw - 1
                with nc.named_scope(f"av{t}"):
                    for i in range(HPC):
                        # emit scores for head pairs together: the odd head's
                        # K=64 matmuls sit at partition offset 64 and can
                        # overlap the even head's in the PE array (row groups)
                        if i % 2 == 0:
                            for j in range(max(0, c0 - 1), min(16, c1 + 2)):
                                for ii in (i, i + 1):
                                    if j not in P[ii]:
                                        emit_scores(ii, j)
                        ps = psA.tile([65, 128 * gw], f32, tag="av", name="psav",
                                      bufs=3, padded_shape=[65, 512])
                        js = [
                            j for j in range(max(0, c0 - 1), min(16, c1 + 2))
                            if max(max(0, j - 1), c0) <= min(min(15, j + 1), c1)
                        ]
                        for n_, j in enumerate(js):
                            jqlo = max(0, j - 1)
                            lo = max(jqlo, c0)
                            hi = min(min(15, j + 1), c1)
                            nc.tensor.matmul(
                                ps[0:65, 128 * (lo - c0):128 * (hi - c0 + 1)],
                                Vv[:, j, i, :],
                                P[i][j][:, 128 * (lo - jqlo):128 * (hi - jqlo + 1)],
                                start=(n_ == 0), stop=(n_ == len(js) - 1),
                                skip_group_check=True,
                            )
                        # normalization
                        rs = spool.tile([1, 128 * gw], f32r, tag="rs", name="rs",
                                        padded_shape=[1, 512])
                        nc.scalar.copy(rs[:], ps[64:65, :])
                        psb = psA.tile([64, 128 * gw], f32, tag="a", name="psb",
                                       padded_shape=[64, 512])
                        nc.tensor.matmul(psb[:], onesb[0:1, :], rs[:],
                                         start=True, stop=True)
                        rec = spool.tile([64, 128 * gw], f32, tag="rec", name="rec",
                                         padded_shape=[64, 512])
                        nc.vector.reciprocal_approx_fast(rec[:], psb[:])
                        pair = i // 2
                        if (pair, t) in out_norm:
                            on = out_norm[(pair, t)]
                        else:
                            on = npool.tile([128, 128 * gw], f32r, tag="on",
                                            name="on", padded_shape=[128, 512])
                            out_norm[(pair, t)] = on
                        r0 = 64 * (i % 2)
                        nc.vector.tensor_mul(on[r0:r0 + 64, :], ps[0:64, :], rec[:])
                with nc.named_scope(f"proj{t}"):
                    for cc in range(gw):
                        c = c0 + cc
                        ps = psA.tile([128, 512], f32, tag="a", name="psa")
                        for pair in range(2):
                            nc.tensor.matmul(
                                ps[:],
                                out_norm[(pair, t)][:, 128 * cc:128 * (cc + 1)],
                                wp[pair][:],
                                start=(pair == 0),
                                stop=(pair == 1),
                            )
                        ob = opool.tile([128, 512], f32, tag="ob", name="ob")
                        nc.vector.tensor_copy(ob[:], ps[:])
                        nc.sync.dma_start(out_d[128 * c:128 * (c + 1), :], ob[:])

    nc.compile()
    return nc


def _get_nc():
    global _NC
    if _NC is None:
        _NC = _build()
    return _NC


def _prep_inputs(x, w_qkv, b_qkv, w_proj):
    mask = _make_mask_strip()
    ident = np.eye(128, dtype=np.float32)
    in_maps = []
    for core in range(N_CORES):
        b, g = core // 2, core % 2
        qc, kc, vc = 256 * g, 512 + 256 * g, 1024 + 256 * g
        wqk = np.concatenate(
            [w_qkv[:, qc:qc + GC] * SCALE, w_qkv[:, kc:kc + GC]], axis=1
        ).astype(np.float32)
        bqk = np.concatenate(
            [b_qkv[qc:qc + GC] * SCALE, b_qkv[kc:kc + GC]]
        ).astype(np.float32)
        in_maps.append({
            "xT": np.ascontiguousarray(x[b].T[:, OLD_OF_NEW], dtype=np.float32),
            "wqk": np.ascontiguousarray(wqk),
            "wv": np.ascontiguousarray(w_qkv[:, vc:vc + GC], dtype=np.float32),
            "wp": np.ascontiguousarray(w_proj[GC * g:GC * (g + 1), :],
                                       dtype=np.float32),
            "bias": np.ascontiguousarray(bqk.reshape(4, 128).T),
            "mask": mask,
            "ident": ident,
            "ones": np.ones((128, 64), np.float32),
        })
    return in_maps


def _assemble(results, b_qkv, b_proj, w_proj):
    const = (b_proj + b_qkv[1024:1536] @ w_proj).astype(np.float32)
    out = np.empty((B, N, DIM), np.float32)
    for b in range(B):
        s = results[2 * b]["out"] + results[2 * b + 1]["out"] + const
        out[b] = s[NEW_OF_OLD]
    return out


def run(x, w_qkv, b_qkv, w_proj, b_proj, trace=False):
    from concourse.bass_utils import run_bass_kernel_spmd

    nc = _get_nc()
    in_maps = _prep_inputs(np.asarray(x), np.asarray(w_qkv),
                           np.asarray(b_qkv), np.asarray(w_proj))
    res = run_bass_kernel_spmd(nc, in_maps, core_ids=list(range(N_CORES)),
                               trace=trace)
    out = _assemble(res.results, np.asarray(b_qkv), np.asarray(b_proj),
                    np.asarray(w_proj))
    return out, res


def kernel(x, w_qkv, b_qkv, w_proj, b_proj):
    out, _ = run(x, w_qkv, b_qkv, w_proj, b_proj, trace=False)
    return out


# revision 39
# speedup vs baseline: 1.0204x; 1.0204x over previous
"""Trainium2 Bass kernel for local-window sparse attention.

Problem (hardcoded):
  x [4, 2048, 512] fp32; qkv proj [512, 1536] + bias; 8 heads, head_dim 64;
  2D local attention on a 16x128 grid with a 7x11 window; out proj [512, 512].

Sharding: 8 cores = 4 batches x 2 head-groups (4 heads each).

Layout trick: queries/keys are permuted to w-major order (idx = w*16 + h) on
the host. Then a 128-query block = 8 w-columns x 16 h-rows, and each query
block attends only key blocks {j-1, j, j+1} (instead of 7 of 16 in h-major
order), and the additive local mask is a single shared [128, 3*128] strip.

Dataflow per core (all matmuls float32r ~= tf32):
  qk^T [512, 2048] = W_qk^T-stationary matmuls over x^T (host-pretransposed)
  V natural [2048, 4*65] (ones column appended per head for rowsums)
  S^T_j [128k, <=384q] = mask-inject (identity matmul) + K_j^T .T @ q^T
  P^T = exp(S^T)               (ScalarE; masked entries underflow to 0)
  outT_aug [65, 512] = sum_j V_aug_j.T @ P^T_j   (row 64 = softmax denoms)
  outT_norm = outT_aug[0:64] * bcast(1/rowsum)   (PE bcast + DVE recip/mult)
  final [128n, 512] = sum_heads outT_norm.T @ W_proj_head  -> DMA out

Host: sums the two head-group partials per batch, adds b_proj + b_v @ w_proj
(v-bias commutes through softmax), and un-permutes rows.
"""

import os

if os.environ.get("JAX_PLATFORMS") == "cpu":
    # The bass kernel needs the neuron/axon jax platform.
    os.environ["JAX_PLATFORMS"] = ""

import numpy as np

DIM = 512
N_HEADS = 8
HEAD_DIM = 64
H, W = 16, 128
HK, WK = 7, 11
B = 4
N = H * W  # 2048
SCALE = HEAD_DIM ** -0.5

N_CORES = 8
HPC = 4  # heads per core
GC = HPC * HEAD_DIM  # 256 feature cols per group
NEG = -60000.0

# w-major permutation: new = w*16 + h  for old = h*128 + w
_old = np.arange(N)
NEW_OF_OLD = (_old % W) * H + (_old // W)
_new = np.arange(N)
OLD_OF_NEW = (_new % H) * W + (_new // H)

GROUPS = [(0, 4), (4, 4), (8, 4), (12, 4)]  # (first chunk, width)

_NC = None  # cached compiled Bass module


def _make_mask_strip() -> np.ndarray:
    """strip[k_local, 128*(r+1) + q_local] for key block j vs query chunk j+r."""
    loc = np.arange(128)
    wl, hl = loc // H, loc % H
    strip = np.full((128, 384), NEG, np.float32)
    for r in (-1, 0, 1):
        dh = hl[:, None] - hl[None, :]
        dw = wl[:, None] - wl[None, :] - 8 * r  # w_k - w_q
        ok = (np.abs(dh) <= HK // 2) & (np.abs(dw) <= WK // 2)
        strip[:, 128 * (r + 1):128 * (r + 2)] = np.where(ok, 0.0, NEG)
    return strip


def _build():
    import concourse.bass as bass
    import concourse.tile as tile
    from concourse import bacc, mybir

    f32 = mybir.dt.float32
    f32r = mybir.dt.float32r
    Exp = mybir.ActivationFunctionType.Exp

    nc = bacc.Bacc("TRN2", target_bir_lowering=False, debug=False)

    xT_d = nc.dram_tensor("xT", [DIM, N], f32r, kind="ExternalInput")
    wqk_d = nc.dram_tensor("wqk", [DIM, 512], f32r, kind="ExternalInput")
    wv_d = nc.dram_tensor("wv", [DIM, GC], f32r, kind="ExternalInput")
    wp_d = nc.dram_tensor("wp", [GC, DIM], f32r, kind="ExternalInput")
    bias_d = nc.dram_tensor("bias", [128, 4], f32, kind="ExternalInput")
    mask_d = nc.dram_tensor("mask", [128, 384], f32r, kind="ExternalInput")
    ident_d = nc.dram_tensor("ident", [128, 128], f32r, kind="ExternalInput")
    ones_d = nc.dram_tensor("ones", [128, 64], f32r, kind="ExternalInput")
    out_d = nc.dram_tensor("out", [N, DIM], f32, kind="ExternalOutput")

    with tile.TileContext(nc) as tc:
        with (
            tc.tile_pool(name="const", bufs=1) as cpool,
            tc.tile_pool(name="big", bufs=1) as bpool,
            tc.tile_pool(name="pexp", bufs=22) as ppool,
            tc.tile_pool(name="onorm", bufs=10) as npool,
            tc.tile_pool(name="small", bufs=6) as spool,
            tc.tile_pool(name="outsb", bufs=4) as opool,
            tc.tile_pool(name="psA", bufs=2, space="PSUM") as psA,
            tc.tile_pool(name="psS", bufs=3, space="PSUM") as psS,
        ):
            # ---- constants / inputs to SBUF (ordered to unblock compute) ----
            wqk = [cpool.tile([128, 512], f32r, tag=f"wqk{d}", name=f"wqk{d}") for d in range(4)]
            wv = [cpool.tile([128, GC], f32r, tag=f"wv{d}", name=f"wv{d}") for d in range(4)]
            wp = [cpool.tile([128, 512], f32r, tag=f"wp{t}", name=f"wp{t}") for t in range(2)]
            xT = [cpool.tile([128, N], f32r, tag=f"xT{d}", name=f"xT{d}") for d in range(4)]
            bias = cpool.tile([128, 4], f32, tag="bias")
            mask = cpool.tile([128, 384], f32r, tag="mask")
            ident = cpool.tile([128, 128], f32r, tag="ident")
            onesb = cpool.tile([128, 64], f32r, tag="onesb")

            # single-ring, just-in-time order: each x^T chunk lands right
            # before its QKV compute needs it (chunk compute ~5us, chunk DMA
            # ~2.8us, so the stream stays ahead after the first chunk)
            for d in range(4):
                nc.sync.dma_start(wqk[d][:], wqk_d[128 * d:128 * (d + 1), :])

            def dma_xt(nchk):
                for d in range(4):
                    nc.sync.dma_start(
                        xT[d][:, 512 * nchk:512 * (nchk + 1)],
                        xT_d[128 * d:128 * (d + 1), 512 * nchk:512 * (nchk + 1)],
                    )

            dma_xt(0)
            nc.sync.dma_start(bias[:], bias_d[:])
            nc.sync.dma_start(mask[:], mask_d[:])
            nc.sync.dma_start(ident[:], ident_d[:])
            nc.sync.dma_start(onesb[:], ones_d[:])
            dma_xt(1)
            for d in range(4):
                nc.sync.dma_start(wv[d][:], wv_d[128 * d:128 * (d + 1), :])
            dma_xt(2)
            for t in range(2):
                nc.sync.dma_start(wp[t][:], wp_d[128 * t:128 * (t + 1), :])
            dma_xt(3)

            # ---- phase B+C interleaved per n-chunk: qk^T and V ----
            qk = [bpool.tile([128, N], f32r, tag=f"qk{t}", name=f"qk{t}") for t in range(4)]
            V = bpool.tile([128, 16 * HPC * 65], f32r, tag="V")
            Vv = V[:].rearrange("p (j h c) -> p j h c", j=16, h=HPC)
            nc.sync.dma_start(
                Vv[:, :, :, 64],
                ones_d[:].rearrange("p (j h) -> p j h", j=16),
            )
            with nc.named_scope("qkv"):
                for nchk in range(4):
                    for ot in range(4):
                        ps = psA.tile([128, 512], f32, tag="a", name="psa")
                        for d in range(4):
                            nc.tensor.matmul(
                                ps[:],
                                wqk[d][:, 128 * ot:128 * (ot + 1)],
                                xT[d][:, 512 * nchk:512 * (nchk + 1)],
                                start=(d == 0),
                                stop=(d == 3),
                            )
                        nc.vector.tensor_scalar_add(
                            qk[ot][:, 512 * nchk:512 * (nchk + 1)],
                            ps[:],
                            bias[:, ot:ot + 1],
                        )
                for nt in range(16):
                    ps = psA.tile([128, GC], f32, tag="a", name="psv",
                                  padded_shape=[128, 512])
                    for d in range(4):
                        nc.tensor.matmul(
                            ps[:],
                            xT[d][:, 128 * nt:128 * (nt + 1)],
                            wv[d][:],
                            start=(d == 0),
                            stop=(d == 3),
                        )
                    nc.vector.tensor_copy(
                        Vv[:, nt, :, 0:64],
                        ps[:].rearrange("p (h c) -> p h c", h=HPC),
                    )

            # helpers for head slicing in qk tiles
            def qT(i):  # [64, N] q^T rows of head i
                return qk[i // 2][64 * (i % 2):64 * (i % 2) + 64, :]

            def kT(i):
                return qk[2 + i // 2][64 * (i % 2):64 * (i % 2) + 64, :]

            P = {i: {} for i in range(HPC)}

            def emit_scores(i, j):
                qlo, qhi = max(0, j - 1), min(15, j + 1)
                wj = 128 * (qhi - qlo + 1)
                mlo = 128 * (qlo - (j - 1))
                ps = psS.tile([128, wj], f32, tag="s", name="pss")
                nc.tensor.matmul(
                    ps[:], ident[:], mask[:, mlo:mlo + wj],
                    start=True, stop=False, skip_group_check=True,
                )
                nc.tensor.matmul(
                    ps[:],
                    kT(i)[:, 128 * j:128 * (j + 1)],
                    qT(i)[:, 128 * qlo:128 * (qhi + 1)],
                    start=False, stop=True, skip_group_check=True,
                )
                pt = ppool.tile([128, wj], f32r, tag="p", name="pt")
                nc.scalar.activation(pt[:], ps[:], Exp)
                P[i][j] = pt

            # ---- t-major: scores -> AV -> normalize -> proj per chunk group
            out_norm = {}  # (pair, group) -> [128, 4*gw] tile
            for t, (c0, gw) in enumerate(GROUPS):
                c1 = c0 + gw - 1
                with nc.named_scope(f"av{t}"):
                    for i in range(HPC):
                        # emit scores for head pairs together: the odd head's
                        # K=64 matmuls sit at partition offset 64 and can
                        # overlap the even head's in the PE array (row groups)
                        if i % 2 == 0:
                            for j in range(max(0, c0 - 1), min(16, c1 + 2)):
                                for ii in (i, i + 1):
                                    if j not in P[ii]:
                                        emit_scores(ii, j)
                        ps = psA.tile([65, 128 * gw], f32, tag="av", name="psav",
                                      bufs=3, padded_shape=[65, 512])
                        js = [
                            j for j in range(max(0, c0 - 1), min(16, c1 + 2))
                            if max(max(0, j - 1), c0) <= min(min(15, j + 1), c1)
                        ]
                        for n_, j in enumerate(js):
                            jqlo = max(0, j - 1)
                            lo = max(jqlo, c0)
                            hi = min(min(15, j + 1), c1)
                            nc.tensor.matmul(
                                ps[0:65, 128 * (lo - c0):128 * (hi - c0 + 1)],
                                Vv[:, j, i, :],
                                P[i][j][:, 128 * (lo - jqlo):128 * (hi - jqlo + 1)],
                                start=(n_ == 0), stop=(n_ == len(js) - 1),
                                skip_group_check=True,
                            )
                        # normalization
                        rs = spool.tile([1, 128 * gw], f32r, tag="rs", name="rs",
                                        padded_shape=[1, 512])
                        nc.scalar.copy(rs[:], ps[64:65, :])
                        psb = psA.tile([64, 128 * gw], f32, tag="a", name="psb",
                                       padded_shape=[64, 512])
                        nc.tensor.matmul(psb[:], onesb[0:1, :], rs[:],
                                         start=True, stop=True)
                        rec = spool.tile([64, 128 * gw], f32, tag="rec", name="rec",
                                         padded_shape=[64, 512])
                        nc.vector.reciprocal_approx_fast(rec[:], psb[:])
                        pair = i // 2
                        if (pair, t) in out_norm:
                            on = out_norm[(pair, t)]
                        else:
                            on = npool.tile([128, 128 * gw], f32r, tag="on",
                                            name="on", padded_shape=[128, 512])
                            out_norm[(pair, t)] = on
                        r0 = 64 * (i % 2)
                        nc.vector.tensor_mul(on[r0:r0 + 64, :], ps[0:64, :], rec[:])
                with nc.named_scope(f"proj{t}"):
                    for cc in range(gw):
                        c = c0 + cc
                        ps = psA.tile([128, 512], f32, tag="a", name="psa")
                        for pair in range(2):
                            nc.tensor.matmul(
                                ps[:],
                                out_norm[(pair, t)][:, 128 * cc:128 * (cc + 1)],
                                wp[pair][:],
                                start=(pair == 0),
                                stop=(pair == 1),
                            )
                        ob = opool.tile([128, 512], f32, tag="ob", name="ob")
                        nc.vector.tensor_copy(ob[:], ps[:])
                        nc.sync.dma_start(out_d[128 * c:128 * (c + 1), :], ob[:])

    nc.compile()
    return nc


def _get_nc():
    global _NC
    if _NC is None:
        _NC = _build()
    return _NC


def _prep_inputs(x, w_qkv, b_qkv, w_proj):
    mask = _make_mask_strip()
    ident = np.eye(128, dtype=np.float32)
    in_maps = []
    for core in range(N_CORES):
        b, g = core // 2, core % 2
        qc, kc, vc = 256 * g, 512 + 256 * g, 1024 + 256 * g
        wqk = np.concatenate(
            [w_qkv[:, qc:qc + GC] * SCALE, w_qkv[:, kc:kc + GC]], axis=1
        ).astype(np.float32)
        bqk = np.concatenate(
            [b_qkv[qc:qc + GC] * SCALE, b_qkv[kc:kc + GC]]
        ).astype(np.float32)
        in_maps.append({
            "xT": np.ascontiguousarray(x[b].T[:, OLD_OF_NEW], dtype=np.float32),
            "wqk": np.ascontiguousarray(wqk),
            "wv": np.ascontiguousarray(w_qkv[:, vc:vc + GC], dtype=np.float32),
            "wp": np.ascontiguousarray(w_proj[GC * g:GC * (g + 1), :],
                                       dtype=np.float32),
            "bias": np.ascontiguousarray(bqk.reshape(4, 128).T),
            "mask": mask,
            "ident": ident,
            "ones": np.ones((128, 64), np.float32),
        })
    return in_maps


def _assemble(results, b_qkv, b_proj, w_proj):
    const = (b_proj + b_qkv[1024:1536] @ w_proj).astype(np.float32)
    out = np.empty((B, N, DIM), np.float32)
    for b in range(B):
        s = results[2 * b]["out"] + results[2 * b + 1]["out"] + const
        out[b] = s[NEW_OF_OLD]
    return out


def run(x, w_qkv, b_qkv, w_proj, b_proj, trace=False):
    from concourse.bass_utils import run_bass_kernel_spmd

    nc = _get_nc()
    in_maps = _prep_inputs(np.asarray(x), np.asarray(w_qkv),
                           np.asarray(b_qkv), np.asarray(w_proj))
    res = run_bass_kernel_spmd(nc, in_maps, core_ids=list(range(N_CORES)),
                               trace=trace)
    out = _assemble(res.results, np.asarray(b_qkv), np.asarray(b_proj),
                    np.asarray(w_proj))
    return out, res


def kernel(x, w_qkv, b_qkv, w_proj, b_proj):
    out, _ = run(x, w_qkv, b_qkv, w_proj, b_proj, trace=False)
    return out


# revision 43
# speedup vs baseline: 1.0223x; 1.0018x over previous
"""Trainium2 Bass kernel for local-window sparse attention.

Problem (hardcoded):
  x [4, 2048, 512] fp32; qkv proj [512, 1536] + bias; 8 heads, head_dim 64;
  2D local attention on a 16x128 grid with a 7x11 window; out proj [512, 512].

Sharding: 8 cores = 4 batches x 2 head-groups (4 heads each).

Layout trick: queries/keys are permuted to w-major order (idx = w*16 + h) on
the host. Then a 128-query block = 8 w-columns x 16 h-rows, and each query
block attends only key blocks {j-1, j, j+1} (instead of 7 of 16 in h-major
order), and the additive local mask is a single shared [128, 3*128] strip.

Dataflow per core (all matmuls float32r ~= tf32):
  qk^T [512, 2048] = W_qk^T-stationary matmuls over x^T (host-pretransposed)
  V natural [2048, 4*65] (ones column appended per head for rowsums)
  S^T_j [128k, <=384q] = mask-inject (identity matmul) + K_j^T .T @ q^T
  P^T = exp(S^T)               (ScalarE; masked entries underflow to 0)
  outT_aug [65, 512] = sum_j V_aug_j.T @ P^T_j   (row 64 = softmax denoms)
  outT_norm = outT_aug[0:64] * bcast(1/rowsum)   (PE bcast + DVE recip/mult)
  final [128n, 512] = sum_heads outT_norm.T @ W_proj_head  -> DMA out

Host: sums the two head-group partials per batch, adds b_proj + b_v @ w_proj
(v-bias commutes through softmax), and un-permutes rows.
"""

import os

if os.environ.get("JAX_PLATFORMS") == "cpu":
    # The bass kernel needs the neuron/axon jax platform.
    os.environ["JAX_PLATFORMS"] = ""

import numpy as np

DIM = 512
N_HEADS = 8
HEAD_DIM = 64
H, W = 16, 128
HK, WK = 7, 11
B = 4
N = H * W  # 2048
SCALE = HEAD_DIM ** -0.5

N_CORES = 8
HPC = 4  # heads per core
GC = HPC * HEAD_DIM  # 256 feature cols per group
NEG = -60000.0

# w-major permutation: new = w*16 + h  for old = h*128 + w
_old = np.arange(N)
NEW_OF_OLD = (_old % W) * H + (_old // W)
_new = np.arange(N)
OLD_OF_NEW = (_new % H) * W + (_new // H)

GROUPS = [(0, 4), (4, 4), (8, 4), (12, 4)]  # (first chunk, width)

_NC = None  # cached compiled Bass module


def _make_mask_strip() -> np.ndarray:
    """strip[k_local, 128*(r+1) + q_local] for key block j vs query chunk j+r."""
    loc = np.arange(128)
    wl, hl = loc // H, loc % H
    strip = np.full((128, 384), NEG, np.float32)
    for r in (-1, 0, 1):
        dh = hl[:, None] - hl[None, :]
        dw = wl[:, None] - wl[None, :] - 8 * r  # w_k - w_q
        ok = (np.abs(dh) <= HK // 2) & (np.abs(dw) <= WK // 2)
        strip[:, 128 * (r + 1):128 * (r + 2)] = np.where(ok, 0.0, NEG)
    return strip


def _build():
    import concourse.bass as bass
    import concourse.tile as tile
    from concourse import bacc, mybir

    f32 = mybir.dt.float32
    f32r = mybir.dt.float32r
    Exp = mybir.ActivationFunctionType.Exp

    nc = bacc.Bacc("TRN2", target_bir_lowering=False, debug=False)

    xT_d = nc.dram_tensor("xT", [DIM, N], f32r, kind="ExternalInput")
    wqk_d = nc.dram_tensor("wqk", [DIM, 512], f32r, kind="ExternalInput")
    wv_d = nc.dram_tensor("wv", [DIM, GC], f32r, kind="ExternalInput")
    wp_d = nc.dram_tensor("wp", [GC, DIM], f32r, kind="ExternalInput")
    bias_d = nc.dram_tensor("bias", [128, 4], f32, kind="ExternalInput")
    mask_d = nc.dram_tensor("mask", [128, 384], f32r, kind="ExternalInput")
    ident_d = nc.dram_tensor("ident", [128, 128], f32r, kind="ExternalInput")
    ones_d = nc.dram_tensor("ones", [128, 64], f32r, kind="ExternalInput")
    out_d = nc.dram_tensor("out", [N, DIM], f32, kind="ExternalOutput")

    with tile.TileContext(nc) as tc:
        with (
            tc.tile_pool(name="const", bufs=1) as cpool,
            tc.tile_pool(name="big", bufs=1) as bpool,
            tc.tile_pool(name="pexp", bufs=14) as ppool,
            tc.tile_pool(name="onorm", bufs=10) as npool,
            tc.tile_pool(name="small", bufs=6) as spool,
            tc.tile_pool(name="outsb", bufs=4) as opool,
            tc.tile_pool(name="psA", bufs=2, space="PSUM") as psA,
            tc.tile_pool(name="psS", bufs=3, space="PSUM") as psS,
        ):
            # ---- constants / inputs to SBUF (ordered to unblock compute) ----
            wqk = [cpool.tile([128, 512], f32r, tag=f"wqk{d}", name=f"wqk{d}") for d in range(4)]
            wv = [cpool.tile([128, GC], f32r, tag=f"wv{d}", name=f"wv{d}") for d in range(4)]
            wp = [cpool.tile([128, 512], f32r, tag=f"wp{t}", name=f"wp{t}") for t in range(2)]
            xT = [cpool.tile([128, N], f32r, tag=f"xT{d}", name=f"xT{d}") for d in range(4)]
            bias = cpool.tile([128, 4], f32, tag="bias")
            mask = cpool.tile([128, 384], f32r, tag="mask")
            ident = cpool.tile([128, 128], f32r, tag="ident")
            onesb = cpool.tile([128, 64], f32r, tag="onesb")

            # single-ring, just-in-time order: each x^T chunk lands right
            # before its QKV compute needs it (chunk compute ~5us, chunk DMA
            # ~2.8us, so the stream stays ahead after the first chunk)
            for d in range(4):
                nc.sync.dma_start(wqk[d][:], wqk_d[128 * d:128 * (d + 1), :])

            def dma_xt(nchk):
                for d in range(4):
                    nc.sync.dma_start(
                        xT[d][:, 512 * nchk:512 * (nchk + 1)],
                        xT_d[128 * d:128 * (d + 1), 512 * nchk:512 * (nchk + 1)],
                    )

            dma_xt(0)
            nc.sync.dma_start(bias[:], bias_d[:])
            nc.sync.dma_start(mask[:], mask_d[:])
            nc.sync.dma_start(ident[:], ident_d[:])
            nc.sync.dma_start(onesb[:], ones_d[:])
            dma_xt(1)
            for d in range(4):
                nc.sync.dma_start(wv[d][:], wv_d[128 * d:128 * (d + 1), :])
            dma_xt(2)
            for t in range(2):
                nc.sync.dma_start(wp[t][:], wp_d[128 * t:128 * (t + 1), :])
            dma_xt(3)

            # ---- phase B+C interleaved per n-chunk: qk^T and V ----
            qk = [bpool.tile([128, N], f32r, tag=f"qk{t}", name=f"qk{t}") for t in range(4)]
            V = bpool.tile([128, 16 * HPC * 65], f32r, tag="V")
            Vv = V[:].rearrange("p (j h c) -> p j h c", j=16, h=HPC)
            nc.sync.dma_start(
                Vv[:, :, :, 64],
                ones_d[:].rearrange("p (j h) -> p j h", j=16),
            )
            with nc.named_scope("qkv"):
                for nchk in range(4):
                    for ot in range(4):
                        ps = psA.tile([128, 512], f32, tag="a", name="psa")
                        for d in range(4):
                            nc.tensor.matmul(
                                ps[:],
                                wqk[d][:, 128 * ot:128 * (ot + 1)],
                                xT[d][:, 512 * nchk:512 * (nchk + 1)],
                                start=(d == 0),
                                stop=(d == 3),
                            )
                        nc.vector.tensor_scalar_add(
                            qk[ot][:, 512 * nchk:512 * (nchk + 1)],
                            ps[:],
                            bias[:, ot:ot + 1],
                        )
                for nt in range(16):
                    ps = psA.tile([128, GC], f32, tag="a", name="psv",
                                  padded_shape=[128, 512])
                    for d in range(4):
                        nc.tensor.matmul(
                            ps[:],
                            xT[d][:, 128 * nt:128 * (nt + 1)],
                            wv[d][:],
                            start=(d == 0),
                            stop=(d == 3),
                        )
                    nc.vector.tensor_copy(
                        Vv[:, nt, :, 0:64],
                        ps[:].rearrange("p (h c) -> p h c", h=HPC),
                    )

            # helpers for head slicing in qk tiles
            def qT(i):  # [64, N] q^T rows of head i
                return qk[i // 2][64 * (i % 2):64 * (i % 2) + 64, :]

            def kT(i):
                return qk[2 + i // 2][64 * (i % 2):64 * (i % 2) + 64, :]

            P = {i: {} for i in range(HPC)}

            def emit_scores(i, j):
                qlo, qhi = max(0, j - 1), min(15, j + 1)
                wj = 128 * (qhi - qlo + 1)
                mlo = 128 * (qlo - (j - 1))
                ps = psS.tile([128, wj], f32, tag="s", name="pss")
                nc.tensor.matmul(
                    ps[:], ident[:], mask[:, mlo:mlo + wj],
                    start=True, stop=False, skip_group_check=True,
                )
                nc.tensor.matmul(
                    ps[:],
                    kT(i)[:, 128 * j:128 * (j + 1)],
                    qT(i)[:, 128 * qlo:128 * (qhi + 1)],
                    start=False, stop=True, skip_group_check=True,
                )
                pt = ppool.tile([128, wj], f32r, tag="p", name="pt")
                nc.scalar.activation(pt[:], ps[:], Exp)
                P[i][j] = pt

            # ---- t-major: scores -> AV -> normalize -> proj per chunk group
            out_norm = {}  # (pair, group) -> [128, 4*gw] tile
            for t, (c0, gw) in enumerate(GROUPS):
                c1 = c0 + gw - 1
                with nc.named_scope(f"av{t}"):
                    for i in range(HPC):
                        # emit scores for head pairs together: the odd head's
                        # K=64 matmuls sit at partition offset 64 and can
                        # overlap the even head's in the PE array (row groups)
                        if i % 2 == 0:
                            for j in range(max(0, c0 - 1), min(16, c1 + 2)):
                                for ii in (i, i + 1):
                                    if j not in P[ii]:
                                        emit_scores(ii, j)
                        ps = psA.tile([65, 128 * gw], f32, tag="av", name="psav",
                                      bufs=3, padded_shape=[65, 512])
                        js = [
                            j for j in range(max(0, c0 - 1), min(16, c1 + 2))
                            if max(max(0, j - 1), c0) <= min(min(15, j + 1), c1)
                        ]
                        for n_, j in enumerate(js):
                            jqlo = max(0, j - 1)
                            lo = max(jqlo, c0)
                            hi = min(min(15, j + 1), c1)
                            nc.tensor.matmul(
                                ps[0:65, 128 * (lo - c0):128 * (hi - c0 + 1)],
                                Vv[:, j, i, :],
                                P[i][j][:, 128 * (lo - jqlo):128 * (hi - jqlo + 1)],
                                start=(n_ == 0), stop=(n_ == len(js) - 1),
                                skip_group_check=True,
                            )
                        # normalization
                        rs = spool.tile([1, 128 * gw], f32r, tag="rs", name="rs",
                                        padded_shape=[1, 512])
                        nc.scalar.copy(rs[:], ps[64:65, :])
                        psb = psA.tile([64, 128 * gw], f32, tag="a", name="psb",
                                       padded_shape=[64, 512])
                        nc.tensor.matmul(psb[:], onesb[0:1, :], rs[:],
                                         start=True, stop=True)
                        rec = spool.tile([64, 128 * gw], f32, tag="rec", name="rec",
                                         padded_shape=[64, 512])
                        nc.vector.reciprocal_approx_fast(rec[:], psb[:])
                        pair = i // 2
                        if (pair, t) in out_norm:
                            on = out_norm[(pair, t)]
                        else:
                            on = npool.tile([128, 128 * gw], f32r, tag="on",
                                            name="on", padded_shape=[128, 512])
                            out_norm[(pair, t)] = on
                        r0 = 64 * (i % 2)
                        nc.vector.tensor_mul(on[r0:r0 + 64, :], ps[0:64, :], rec[:])
                with nc.named_scope(f"proj{t}"):
                    for cc in range(gw):
                        c = c0 + cc
                        ps = psA.tile([128, 512], f32, tag="a", name="psa")
                        for pair in range(2):
                            nc.tensor.matmul(
                                ps[:],
                                out_norm[(pair, t)][:, 128 * cc:128 * (cc + 1)],
                                wp[pair][:],
                                start=(pair == 0),
                                stop=(pair == 1),
                            )
                        ob = opool.tile([128, 512], f32, tag="ob", name="ob")
                        nc.vector.tensor_copy(ob[:], ps[:])
                        nc.sync.dma_start(out_d[128 * c:128 * (c + 1), :], ob[:])

    nc.compile()
    return nc


def _get_nc():
    global _NC
    if _NC is None:
        _NC = _build()
    return _NC


def _prep_inputs(x, w_qkv, b_qkv, w_proj):
    mask = _make_mask_strip()
    ident = np.eye(128, dtype=np.float32)
    in_maps = []
    for core in range(N_CORES):
        b, g = core // 2, core % 2
        qc, kc, vc = 256 * g, 512 + 256 * g, 1024 + 256 * g
        wqk = np.concatenate(
            [w_qkv[:, qc:qc + GC] * SCALE, w_qkv[:, kc:kc + GC]], axis=1
        ).astype(np.float32)
        bqk = np.concatenate(
            [b_qkv[qc:qc + GC] * SCALE, b_qkv[kc:kc + GC]]
        ).astype(np.float32)
        in_maps.append({
            "xT": np.ascontiguousarray(x[b].T[:, OLD_OF_NEW], dtype=np.float32),
            "wqk": np.ascontiguousarray(wqk),
            "wv": np.ascontiguousarray(w_qkv[:, vc:vc + GC], dtype=np.float32),
            "wp": np.ascontiguousarray(w_proj[GC * g:GC * (g + 1), :],
                                       dtype=np.float32),
            "bias": np.ascontiguousarray(bqk.reshape(4, 128).T),
            "mask": mask,
            "ident": ident,
            "ones": np.ones((128, 64), np.float32),
        })
    return in_maps


def _assemble(results, b_qkv, b_proj, w_proj):
    const = (b_proj + b_qkv[1024:1536] @ w_proj).astype(np.float32)
    out = np.empty((B, N, DIM), np.float32)
    for b in range(B):
        s = results[2 * b]["out"] + results[2 * b + 1]["out"] + const
        out[b] = s[NEW_OF_OLD]
    return out


def run(x, w_qkv, b_qkv, w_proj, b_proj, trace=False):
    from concourse.bass_utils import run_bass_kernel_spmd

    nc = _get_nc()
    in_maps = _prep_inputs(np.asarray(x), np.asarray(w_qkv),
                           np.asarray(b_qkv), np.asarray(w_proj))
    res = run_bass_kernel_spmd(nc, in_maps, core_ids=list(range(N_CORES)),
                               trace=trace)
    out = _assemble(res.results, np.asarray(b_qkv), np.asarray(b_proj),
                    np.asarray(w_proj))
    return out, res


def kernel(x, w_qkv, b_qkv, w_proj, b_proj):
    out, _ = run(x, w_qkv, b_qkv, w_proj, b_proj, trace=False)
    return out


# revision 44
# speedup vs baseline: 1.0449x; 1.0221x over previous
"""Trainium2 Bass kernel for local-window sparse attention.

Problem (hardcoded):
  x [4, 2048, 512] fp32; qkv proj [512, 1536] + bias; 8 heads, head_dim 64;
  2D local attention on a 16x128 grid with a 7x11 window; out proj [512, 512].

Sharding: 8 cores = 4 batches x 2 head-groups (4 heads each).

Layout trick: queries/keys are permuted to w-major order (idx = w*16 + h) on
the host. Then a 128-query block = 8 w-columns x 16 h-rows, and each query
block attends only key blocks {j-1, j, j+1} (instead of 7 of 16 in h-major
order), and the additive local mask is a single shared [128, 3*128] strip.

Dataflow per core (all matmuls float32r ~= tf32):
  qk^T [512, 2048] = W_qk^T-stationary matmuls over x^T (host-pretransposed)
  V natural [2048, 4*65] (ones column appended per head for rowsums)
  S^T_j [128k, <=384q] = mask-inject (identity matmul) + K_j^T .T @ q^T
  P^T = exp(S^T)               (ScalarE; masked entries underflow to 0)
  outT_aug [65, 512] = sum_j V_aug_j.T @ P^T_j   (row 64 = softmax denoms)
  outT_norm = outT_aug[0:64] * bcast(1/rowsum)   (PE bcast + DVE recip/mult)
  final [128n, 512] = sum_heads outT_norm.T @ W_proj_head  -> DMA out

Host: sums the two head-group partials per batch, adds b_proj + b_v @ w_proj
(v-bias commutes through softmax), and un-permutes rows.
"""

import os

if os.environ.get("JAX_PLATFORMS") == "cpu":
    # The bass kernel needs the neuron/axon jax platform.
    os.environ["JAX_PLATFORMS"] = ""

import numpy as np

DIM = 512
N_HEADS = 8
HEAD_DIM = 64
H, W = 16, 128
HK, WK = 7, 11
B = 4
N = H * W  # 2048
SCALE = HEAD_DIM ** -0.5

N_CORES = 8
HPC = 4  # heads per core
GC = HPC * HEAD_DIM  # 256 feature cols per group
NEG = -60000.0

# w-major permutation: new = w*16 + h  for old = h*128 + w
_old = np.arange(N)
NEW_OF_OLD = (_old % W) * H + (_old // W)
_new = np.arange(N)
OLD_OF_NEW = (_new % H) * W + (_new // H)

GROUPS = [(0, 4), (4, 4), (8, 4), (12, 4)]  # (first chunk, width)

_NC = None  # cached compiled Bass module


def _make_mask_strip() -> np.ndarray:
    """strip[k_local, 128*(r+1) + q_local] for key block j vs query chunk j+r."""
    loc = np.arange(128)
    wl, hl = loc // H, loc % H
    strip = np.full((128, 384), NEG, np.float32)
    for r in (-1, 0, 1):
        dh = hl[:, None] - hl[None, :]
        dw = wl[:, None] - wl[None, :] - 8 * r  # w_k - w_q
        ok = (np.abs(dh) <= HK // 2) & (np.abs(dw) <= WK // 2)
        strip[:, 128 * (r + 1):128 * (r + 2)] = np.where(ok, 0.0, NEG)
    return strip


def _build():
    import concourse.bass as bass
    import concourse.tile as tile
    from concourse import bacc, mybir

    f32 = mybir.dt.float32
    f32r = mybir.dt.float32r
    Exp = mybir.ActivationFunctionType.Exp

    nc = bacc.Bacc("TRN2", target_bir_lowering=False, debug=False)

    xT_d = nc.dram_tensor("xT", [DIM, N], f32r, kind="ExternalInput")
    wqk_d = nc.dram_tensor("wqk", [DIM, 512], f32r, kind="ExternalInput")
    wv_d = nc.dram_tensor("wv", [DIM, GC], f32r, kind="ExternalInput")
    wp_d = nc.dram_tensor("wp", [GC, DIM], f32r, kind="ExternalInput")
    bias_d = nc.dram_tensor("bias", [128, 4], f32, kind="ExternalInput")
    mask_d = nc.dram_tensor("mask", [128, 384], f32r, kind="ExternalInput")
    ident_d = nc.dram_tensor("ident", [128, 128], f32r, kind="ExternalInput")
    ones_d = nc.dram_tensor("ones", [128, 64], f32r, kind="ExternalInput")
    out_d = nc.dram_tensor("out", [N, DIM], f32, kind="ExternalOutput")

    with tile.TileContext(nc) as tc:
        with (
            tc.tile_pool(name="const", bufs=1) as cpool,
            tc.tile_pool(name="big", bufs=1) as bpool,
            tc.tile_pool(name="pexp", bufs=14) as ppool,
            tc.tile_pool(name="onorm", bufs=10) as npool,
            tc.tile_pool(name="small", bufs=6) as spool,
            tc.tile_pool(name="outsb", bufs=4) as opool,
            tc.tile_pool(name="psA", bufs=2, space="PSUM") as psA,
            tc.tile_pool(name="psS", bufs=3, space="PSUM") as psS,
        ):
            # ---- constants / inputs to SBUF (ordered to unblock compute) ----
            wqk = [cpool.tile([128, 512], f32r, tag=f"wqk{d}", name=f"wqk{d}") for d in range(4)]
            wv = [cpool.tile([128, GC], f32r, tag=f"wv{d}", name=f"wv{d}") for d in range(4)]
            wp = [cpool.tile([128, 512], f32r, tag=f"wp{t}", name=f"wp{t}") for t in range(2)]
            xT = [cpool.tile([128, N], f32r, tag=f"xT{d}", name=f"xT{d}") for d in range(4)]
            bias = cpool.tile([128, 4], f32, tag="bias")
            mask = cpool.tile([128, 384], f32r, tag="mask")
            ident = cpool.tile([128, 128], f32r, tag="ident")
            onesb = cpool.tile([128, 64], f32r, tag="onesb")

            # single-ring, just-in-time order: each x^T chunk lands right
            # before its QKV compute needs it (chunk compute ~5us, chunk DMA
            # ~2.8us, so the stream stays ahead after the first chunk)
            def dma_xt(nchk):
                for d in range(4):
                    nc.sync.dma_start(
                        xT[d][:, 512 * nchk:512 * (nchk + 1)],
                        xT_d[128 * d:128 * (d + 1), 512 * nchk:512 * (nchk + 1)],
                    )

            # interleave wqk[d] with xT[d] chunk0 so the d-accumulation
            # matmuls start as soon as each pair lands
            for d in range(4):
                nc.sync.dma_start(wqk[d][:], wqk_d[128 * d:128 * (d + 1), :])
                nc.sync.dma_start(xT[d][:, 0:512],
                                  xT_d[128 * d:128 * (d + 1), 0:512])
            nc.sync.dma_start(bias[:], bias_d[:])
            nc.sync.dma_start(mask[:], mask_d[:])
            nc.sync.dma_start(ident[:], ident_d[:])
            nc.sync.dma_start(onesb[:], ones_d[:])
            dma_xt(1)
            for d in range(4):
                nc.sync.dma_start(wv[d][:], wv_d[128 * d:128 * (d + 1), :])
            dma_xt(2)
            for t in range(2):
                nc.sync.dma_start(wp[t][:], wp_d[128 * t:128 * (t + 1), :])
            dma_xt(3)

            # ---- phase B+C interleaved per n-chunk: qk^T and V ----
            qk = [bpool.tile([128, N], f32r, tag=f"qk{t}", name=f"qk{t}") for t in range(4)]
            V = bpool.tile([128, 16 * HPC * 65], f32r, tag="V")
            Vv = V[:].rearrange("p (j h c) -> p j h c", j=16, h=HPC)
            nc.sync.dma_start(
                Vv[:, :, :, 64],
                ones_d[:].rearrange("p (j h) -> p j h", j=16),
            )
            with nc.named_scope("qkv"):
                for nchk in range(4):
                    for ot in range(4):
                        ps = psA.tile([128, 512], f32, tag="a", name="psa")
                        for d in range(4):
                            nc.tensor.matmul(
                                ps[:],
                                wqk[d][:, 128 * ot:128 * (ot + 1)],
                                xT[d][:, 512 * nchk:512 * (nchk + 1)],
                                start=(d == 0),
                                stop=(d == 3),
                            )
                        nc.vector.tensor_scalar_add(
                            qk[ot][:, 512 * nchk:512 * (nchk + 1)],
                            ps[:],
                            bias[:, ot:ot + 1],
                        )
                for nt in range(16):
                    ps = psA.tile([128, GC], f32, tag="a", name="psv",
                                  padded_shape=[128, 512])
                    for d in range(4):
                        nc.tensor.matmul(
                            ps[:],
                            xT[d][:, 128 * nt:128 * (nt + 1)],
                            wv[d][:],
                            start=(d == 0),
                            stop=(d == 3),
                        )
                    nc.vector.tensor_copy(
                        Vv[:, nt, :, 0:64],
                        ps[:].rearrange("p (h c) -> p h c", h=HPC),
                    )

            # helpers for head slicing in qk tiles
            def qT(i):  # [64, N] q^T rows of head i
                return qk[i // 2][64 * (i % 2):64 * (i % 2) + 64, :]

            def kT(i):
                return qk[2 + i // 2][64 * (i % 2):64 * (i % 2) + 64, :]

            P = {i: {} for i in range(HPC)}

            def emit_scores(i, j):
                qlo, qhi = max(0, j - 1), min(15, j + 1)
                wj = 128 * (qhi - qlo + 1)
                mlo = 128 * (qlo - (j - 1))
                ps = psS.tile([128, wj], f32, tag="s", name="pss")
                nc.tensor.matmul(
                    ps[:], ident[:], mask[:, mlo:mlo + wj],
                    start=True, stop=False, skip_group_check=True,
                )
                nc.tensor.matmul(
                    ps[:],
                    kT(i)[:, 128 * j:128 * (j + 1)],
                    qT(i)[:, 128 * qlo:128 * (qhi + 1)],
                    start=False, stop=True, skip_group_check=True,
                )
                pt = ppool.tile([128, wj], f32r, tag="p", name="pt")
                nc.scalar.activation(pt[:], ps[:], Exp)
                P[i][j] = pt

            # ---- t-major: scores -> AV -> normalize -> proj per chunk group
            out_norm = {}  # (pair, group) -> [128, 4*gw] tile
            for t, (c0, gw) in enumerate(GROUPS):
                c1 = c0 + gw - 1
                with nc.named_scope(f"av{t}"):
                    for i in range(HPC):
                        # emit scores for head pairs together: the odd head's
                        # K=64 matmuls sit at partition offset 64 and can
                        # overlap the even head's in the PE array (row groups)
                        if i % 2 == 0:
                            for j in range(max(0, c0 - 1), min(16, c1 + 2)):
                                for ii in (i, i + 1):
                                    if j not in P[ii]:
                                        emit_scores(ii, j)
                        ps = psA.tile([65, 128 * gw], f32, tag="av", name="psav",
                                      bufs=3, padded_shape=[65, 512])
                        js = [
                            j for j in range(max(0, c0 - 1), min(16, c1 + 2))
                            if max(max(0, j - 1), c0) <= min(min(15, j + 1), c1)
                        ]
                        for n_, j in enumerate(js):
                            jqlo = max(0, j - 1)
                            lo = max(jqlo, c0)
                            hi = min(min(15, j + 1), c1)
                            nc.tensor.matmul(
                                ps[0:65, 128 * (lo - c0):128 * (hi - c0 + 1)],
                                Vv[:, j, i, :],
                                P[i][j][:, 128 * (lo - jqlo):128 * (hi - jqlo + 1)],
                                start=(n_ == 0), stop=(n_ == len(js) - 1),
                                skip_group_check=True,
                            )
                        # normalization
                        rs = spool.tile([1, 128 * gw], f32r, tag="rs", name="rs",
                                        padded_shape=[1, 512])
                        nc.scalar.copy(rs[:], ps[64:65, :])
                        psb = psA.tile([64, 128 * gw], f32, tag="a", name="psb",
                                       padded_shape=[64, 512])
                        nc.tensor.matmul(psb[:], onesb[0:1, :], rs[:],
                                         start=True, stop=True)
                        rec = spool.tile([64, 128 * gw], f32, tag="rec", name="rec",
                                         padded_shape=[64, 512])
                        nc.vector.reciprocal_approx_fast(rec[:], psb[:])
                        pair = i // 2
                        if (pair, t) in out_norm:
                            on = out_norm[(pair, t)]
                        else:
                            on = npool.tile([128, 128 * gw], f32r, tag="on",
                                            name="on", padded_shape=[128, 512])
                            out_norm[(pair, t)] = on
                        r0 = 64 * (i % 2)
                        nc.vector.tensor_mul(on[r0:r0 + 64, :], ps[0:64, :], rec[:])
                with nc.named_scope(f"proj{t}"):
                    for cc in range(gw):
                        c = c0 + cc
                        ps = psA.tile([128, 512], f32, tag="a", name="psa")
                        for pair in range(2):
                            nc.tensor.matmul(
                                ps[:],
                                out_norm[(pair, t)][:, 128 * cc:128 * (cc + 1)],
                                wp[pair][:],
                                start=(pair == 0),
                                stop=(pair == 1),
                            )
                        ob = opool.tile([128, 512], f32, tag="ob", name="ob")
                        nc.vector.tensor_copy(ob[:], ps[:])
                        nc.sync.dma_start(out_d[128 * c:128 * (c + 1), :], ob[:])

    nc.compile()
    return nc


def _get_nc():
    global _NC
    if _NC is None:
        _NC = _build()
    return _NC


def _prep_inputs(x, w_qkv, b_qkv, w_proj):
    mask = _make_mask_strip()
    ident = np.eye(128, dtype=np.float32)
    in_maps = []
    for core in range(N_CORES):
        b, g = core // 2, core % 2
        qc, kc, vc = 256 * g, 512 + 256 * g, 1024 + 256 * g
        wqk = np.concatenate(
            [w_qkv[:, qc:qc + GC] * SCALE, w_qkv[:, kc:kc + GC]], axis=1
        ).astype(np.float32)
        bqk = np.concatenate(
            [b_qkv[qc:qc + GC] * SCALE, b_qkv[kc:kc + GC]]
        ).astype(np.float32)
        in_maps.append({
            "xT": np.ascontiguousarray(x[b].T[:, OLD_OF_NEW], dtype=np.float32),
            "wqk": np.ascontiguousarray(wqk),
            "wv": np.ascontiguousarray(w_qkv[:, vc:vc + GC], dtype=np.float32),
            "wp": np.ascontiguousarray(w_proj[GC * g:GC * (g + 1), :],
                                       dtype=np.float32),
            "bias": np.ascontiguousarray(bqk.reshape(4, 128).T),
            "mask": mask,
            "ident": ident,
            "ones": np.ones((128, 64), np.float32),
        })
    return in_maps


def _assemble(results, b_qkv, b_proj, w_proj):
    const = (b_proj + b_qkv[1024:1536] @ w_proj).astype(np.float32)
    out = np.empty((B, N, DIM), np.float32)
    for b in range(B):
        s = results[2 * b]["out"] + results[2 * b + 1]["out"] + const
        out[b] = s[NEW_OF_OLD]
    return out


def run(x, w_qkv, b_qkv, w_proj, b_proj, trace=False):
    from concourse.bass_utils import run_bass_kernel_spmd

    nc = _get_nc()
    in_maps = _prep_inputs(np.asarray(x), np.asarray(w_qkv),
                           np.asarray(b_qkv), np.asarray(w_proj))
    res = run_bass_kernel_spmd(nc, in_maps, core_ids=list(range(N_CORES)),
                               trace=trace)
    out = _assemble(res.results, np.asarray(b_qkv), np.asarray(b_proj),
                    np.asarray(w_proj))
    return out, res


def kernel(x, w_qkv, b_qkv, w_proj, b_proj):
    out, _ = run(x, w_qkv, b_qkv, w_proj, b_proj, trace=False)
    return out
